# revision 9
# baseline (speedup 1.0000x reference)
"""Trainium2 kernel for the damped-spring (DMP-style) batched scan.

Reference semantics (per batch b, dof n, x0=dx0=0):
    ddx_t = ax*(bx*(goal - x_t) - dx_t) + f_t
    dx += ddx_t*DT;  x += dx*DT;  traj[..., t] = x

Linear time-invariant 2nd-order recurrence in s=(x,dx):
    s_{t+1} = A s_t + v*u_t,   u_t = f_t + ax*bx*goal,  v = (DT^2, DT)
so traj splits into two independent parts:
    traj[b,n,t] = conv(h, force[b,n,:])[t]  +  goal[b,n] * g(t)
with h(k) = [A^k v]_0 the x-impulse response and g(t) = ax*bx*cumsum(h).
For ax=25, bx=6.25 the poles are 0.912/0.822, so h decays below 1e-5 of
its peak within 128 taps: the sequential scan becomes a SHORT CAUSAL
CONVOLUTION (128 taps), i.e. pure tensor-engine matmuls with no serial
dependency at all.

Device computes only the force part (99.9% of the FLOPs, ~0.1% of the
output norm), tiled per 128-time-block as
    psum[j,f] = sum_i H0[j,i] f_blk[i,f] + sum_i H1[j,i] f_prev[i,f]
with f pre-transposed to [T, F] on the host so time is the contraction
(partition) axis.  Both taps matrices are fused into ONE fp8 DoubleRow
matmul per 512-seq chunk (K=256 effective contraction, 2 fp8 weights per
PE cell): f blocks live in a 3-slot SBUF ring so (cur, prev) sit at a
fixed positive stride, with [H1|H0]-swapped weights covering the ring
wrap.  The exactly-known rank-1 goal part goal*g(t) is added back on the
host in fp64.  fp8 e4m3 in / fp8 out keeps DMA at 32 MB/core (vs 128 MB
for the fp32 scan baseline); fp32 PSUM accumulation keeps the end-to-end
relative L2 error ~6e-5 (tolerance 2e-2).  All values are kept inside
+-240 (TRN fp8e4 max normal).

Sharding: data-parallel over batch across 8 cores; core c takes batches
[256c, 256c+256) = 4096 sequences, each core fully independent.
"""

import os
import numpy as np

_B, _N, _T = 2048, 16, 4096
_NCORES = 8
_P = 128
_SEQ = (_B // _NCORES) * _N          # 4096 sequences per core
_NBLK = _T // _P                     # 32 time blocks
_DT = float(np.float32(0.01))
_S_H = float(2.0 ** 18)              # fp8 scale on the filter taps
_S_OUT = float(2.0 ** 8)             # fp8 scale on the output (force part)

LAST_RESULT = None                   # BassKernelResults stash for harnesses


def _impulse(ax: float, bx: float, n: int):
    """fp64 impulse response h[k] = [A^k v]_0 of the discrete recurrence."""
    a, b, dt = float(ax), float(bx), _DT
    A = np.array(
        [[1.0 - a * b * dt * dt, dt * (1.0 - a * dt)],
         [-a * b * dt, 1.0 - a * dt]], dtype=np.float64)
    v = np.array([dt * dt, dt], dtype=np.float64)
    h = np.empty(n, dtype=np.float64)
    w = v.copy()
    for k in range(n):
        h[k] = w[0]
        w = A @ w
    return h


def _kernel_numpy(force, goal, ax, bx):
    """Exact fallback (slow): used only if the taps don't decay fast."""
    B, N, T = force.shape
    dt = np.float32(_DT)
    x = np.zeros((B, N), np.float32)
    dx = np.zeros((B, N), np.float32)
    out = np.empty((B, N, T), np.float32)
    axf, bxf = np.float32(ax), np.float32(bx)
    for t in range(T):
        ddx = axf * (bxf * (goal - x) - dx) + force[:, :, t]
        dx = dx + ddx * dt
        x = x + dx * dt
        out[:, :, t] = x
    return out


def _build_program():
    import concourse.bacc as bacc
    import concourse.mybir as mybir
    from concourse.tile import TileContext
    from concourse.ap import AP

    f32 = mybir.dt.float32
    f8 = mybir.dt.float8e4
    ident = mybir.ActivationFunctionType.Identity
    DR = mybir.MatmulPerfMode.DoubleRow
    SC = _S_OUT / _S_H
    HALF = _SEQ // 2                 # 2048
    SLOT = _SEQ                      # ring slot stride (elements)

    nc = bacc.Bacc()
    f_d = nc.declare_dram_parameter("f", [_T, _SEQ], f8, isOutput=False)
    # [H0T|H1T] and [H1T|H0T] stacked along free dim (DoubleRow k-tiles)
    w01_d = nc.declare_dram_parameter("w01", [_P, 2 * _P], f8, isOutput=False)
    w10_d = nc.declare_dram_parameter("w10", [_P, 2 * _P], f8, isOutput=False)
    out_d = nc.declare_dram_parameter("out", [_T, _SEQ], f8, isOutput=True)

    with TileContext(nc) as tc:
        with tc.tile_pool(name="const", bufs=1) as cpool, \
             tc.tile_pool(name="oout", bufs=4) as opool, \
             tc.tile_pool(name="ps", bufs=4, space="PSUM") as pspool:
            w01_t = cpool.tile([_P, 2 * _P], f8, tag="w01")
            nc.sync.dma_start(out=w01_t[:], in_=w01_d[:, :])
            w10_t = cpool.tile([_P, 2 * _P], f8, tag="w10")
            nc.sync.dma_start(out=w10_t[:], in_=w10_d[:, :])
            w01 = w01_t[:, :].rearrange("p (two m) -> p two m", two=2)
            w10 = w10_t[:, :].rearrange("p (two m) -> p two m", two=2)

            # 4-slot input ring: (cur, prev) blocks at fixed free-dim offsets
            NRING = 4
            fring = cpool.tile([_P, NRING * SLOT], f8, tag="fring")

            QW = _SEQ // 4           # 1024-wide psum quarter tiles
            for k in range(_NBLK):
                rows = slice(k * _P, (k + 1) * _P)
                s_cur = (k % NRING) * SLOT
                s_prev = ((k - 1) % NRING) * SLOT
                # input loads on SWDGE (GpSimd) - leaves ACT free for evicts
                nc.gpsimd.dma_start(out=fring[:, s_cur:s_cur + SLOT],
                                    in_=f_d[rows, :])
                # ascending (base, base+stride) pair selects which weight
                # order pairs with (cur, prev) in the DoubleRow contraction
                if s_cur < s_prev:
                    base, stride, w3d = s_cur, s_prev - s_cur, w01
                else:
                    base, stride, w3d = s_prev, s_cur - s_prev, w10
                proto = fring[:, 0:1]
                o_t = opool.tile([_P, _SEQ], f8, tag="o")
                for q in range(4):
                    qb = q * QW
                    ps = pspool.tile([_P, QW], f32, tag="ps")
                    for c in range(2):
                        sl = slice(qb + c * 512, qb + (c + 1) * 512)
                        if k == 0:
                            # no previous block: plain matmul on H0T only
                            nc.tensor.matmul(ps[:, c * 512:(c + 1) * 512],
                                             w01_t[:, 0:_P],
                                             fring[:, sl],
                                             start=True, stop=True)
                        else:
                            off = base + qb + (c * 512)
                            rhs = AP(proto.tensor, proto.offset + off,
                                     [list(proto.ap[0]), [stride, 2],
                                      [1, 512]])
                            nc.tensor.matmul(ps[:, c * 512:(c + 1) * 512],
                                             w3d, rhs,
                                             start=True, stop=True,
                                             perf_mode=DR)
                    # evict PSUM->SBUF with the fp8 rescale; one op per
                    # quarter, alternating the otherwise-idle ACT / DVE
                    if q % 2 == 0:
                        nc.scalar.activation(o_t[:, qb:qb + QW],
                                             ps[:, :], ident,
                                             bias=0.0, scale=SC)
                    else:
                        nc.vector.tensor_scalar_mul(o_t[:, qb:qb + QW],
                                                    ps[:, :], SC)
                nc.sync.dma_start(out=out_d[rows, :], in_=o_t[:])
    nc.compile()
    return nc


def kernel(force, goal, ax, bx):
    global LAST_RESULT
    import ml_dtypes

    force = np.asarray(force, dtype=np.float32)
    goal = np.asarray(goal, dtype=np.float32)
    assert force.shape == (_B, _N, _T), force.shape

    h = _impulse(float(ax), float(bx), _T)
    # Fast path needs the taps beyond 128 to be negligible.
    hn = np.linalg.norm(h)
    if not np.isfinite(hn) or hn == 0.0 or \
            np.linalg.norm(h[_P:]) / hn > 1e-3:
        return _kernel_numpy(force, goal, ax, bx)

    f8 = ml_dtypes.float8_e4m3fn

    # Filter matrices (lhsT layout: [K=i(time-in), M=j(time-out)]).
    idx_j, idx_i = np.meshgrid(np.arange(_P), np.arange(_P), indexing="xy")
    # h0t[i, j] = h[j-i] for j>=i      (intra-block taps 0..127)
    lag0 = idx_j - idx_i
    h0t = np.where(lag0 >= 0, h[np.clip(lag0, 0, _P - 1)], 0.0) * _S_H
    # h1t[i, j] = h[j+128-i] for lag<128 (previous-block taps 1..127)
    lag1 = idx_j + _P - idx_i
    h1t = np.where(lag1 < _P, h[np.clip(lag1, 0, _P - 1)], 0.0) * _S_H
    h0t = np.ascontiguousarray(h0t, dtype=np.float32).astype(f8)
    h1t = np.ascontiguousarray(h1t, dtype=np.float32).astype(f8)
    w01 = np.ascontiguousarray(np.concatenate([h0t, h1t], axis=1))
    w10 = np.ascontiguousarray(np.concatenate([h1t, h0t], axis=1))

    nc = _build_program()

    # Shard: core c gets batches [256c, 256c+256) -> [T, SEQ] fp8, transposed
    fq = force.reshape(_NCORES, _SEQ, _T).astype(f8)
    in_maps = [
        {
            "f": np.ascontiguousarray(fq[c].T),
            "w01": w01,
            "w10": w10,
        }
        for c in range(_NCORES)
    ]

    from concourse.bass_utils import run_bass_kernel_spmd
    res = run_bass_kernel_spmd(
        nc, in_maps, list(range(_NCORES)),
        trace=bool(os.environ.get("KERNEL_TRACE")),
    )
    LAST_RESULT = res

    # Host reconstruction: force part (device) + rank-1 goal part (exact).
    g = (float(ax) * float(bx)) * np.cumsum(h)          # (T,) fp64
    out = np.empty((_B, _N, _T), dtype=np.float32)
    ov = out.reshape(_NCORES, _SEQ, _T)
    inv = np.float32(1.0 / _S_OUT)
    gp32 = g.astype(np.float32)
    goal_v = goal.reshape(_NCORES, _SEQ)
    for c in range(_NCORES):
        dev = res.results[c]["out"].astype(np.float32).T   # (SEQ, T)
        np.multiply(dev, inv, out=dev)
        dev += goal_v[c][:, None] * gp32[None, :]
        ov[c] = dev
    return out


# revision 10
# speedup vs baseline: 1.0761x; 1.0761x over previous
"""Trainium2 kernel for the damped-spring (DMP-style) batched scan.

Reference semantics (per batch b, dof n, x0=dx0=0):
    ddx_t = ax*(bx*(goal - x_t) - dx_t) + f_t
    dx += ddx_t*DT;  x += dx*DT;  traj[..., t] = x

Linear time-invariant 2nd-order recurrence in s=(x,dx):
    s_{t+1} = A s_t + v*u_t,   u_t = f_t + ax*bx*goal,  v = (DT^2, DT)
so traj splits into two independent parts:
    traj[b,n,t] = conv(h, force[b,n,:])[t]  +  goal[b,n] * g(t)
with h(k) = [A^k v]_0 the x-impulse response and g(t) = ax*bx*cumsum(h).
For ax=25, bx=6.25 the poles are 0.912/0.822, so h decays below 1e-5 of
its peak within 128 taps: the sequential scan becomes a SHORT CAUSAL
CONVOLUTION (128 taps), i.e. pure tensor-engine matmuls with no serial
dependency at all.

Device computes only the force part (99.9% of the FLOPs, ~0.1% of the
output norm), tiled per 128-time-block as
    psum[j,f] = sum_i H0[j,i] f_blk[i,f] + sum_i H1[j,i] f_prev[i,f]
with f pre-transposed to [T, F] on the host so time is the contraction
(partition) axis.  Both taps matrices are fused into ONE fp8 DoubleRow
matmul per 512-seq chunk (K=256 effective contraction, 2 fp8 weights per
PE cell): f blocks live in a 3-slot SBUF ring so (cur, prev) sit at a
fixed positive stride, with [H1|H0]-swapped weights covering the ring
wrap.  The exactly-known rank-1 goal part goal*g(t) is added back on the
host in fp64.  fp8 e4m3 in / fp8 out keeps DMA at 32 MB/core (vs 128 MB
for the fp32 scan baseline); fp32 PSUM accumulation keeps the end-to-end
relative L2 error ~6e-5 (tolerance 2e-2).  All values are kept inside
+-240 (TRN fp8e4 max normal).

Sharding: data-parallel over batch across 8 cores; core c takes batches
[256c, 256c+256) = 4096 sequences, each core fully independent.
"""

import os
import numpy as np

_B, _N, _T = 2048, 16, 4096
_NCORES = 8
_P = 128
_SEQ = (_B // _NCORES) * _N          # 4096 sequences per core
_NBLK = _T // _P                     # 32 time blocks
_DT = float(np.float32(0.01))
_S_H = float(2.0 ** 18)              # fp8 scale on the filter taps
_S_OUT = float(2.0 ** 8)             # fp8 scale on the output (force part)

LAST_RESULT = None                   # BassKernelResults stash for harnesses


def _impulse(ax: float, bx: float, n: int):
    """fp64 impulse response h[k] = [A^k v]_0 of the discrete recurrence."""
    a, b, dt = float(ax), float(bx), _DT
    A = np.array(
        [[1.0 - a * b * dt * dt, dt * (1.0 - a * dt)],
         [-a * b * dt, 1.0 - a * dt]], dtype=np.float64)
    v = np.array([dt * dt, dt], dtype=np.float64)
    h = np.empty(n, dtype=np.float64)
    w = v.copy()
    for k in range(n):
        h[k] = w[0]
        w = A @ w
    return h


def _kernel_numpy(force, goal, ax, bx):
    """Exact fallback (slow): used only if the taps don't decay fast."""
    B, N, T = force.shape
    dt = np.float32(_DT)
    x = np.zeros((B, N), np.float32)
    dx = np.zeros((B, N), np.float32)
    out = np.empty((B, N, T), np.float32)
    axf, bxf = np.float32(ax), np.float32(bx)
    for t in range(T):
        ddx = axf * (bxf * (goal - x) - dx) + force[:, :, t]
        dx = dx + ddx * dt
        x = x + dx * dt
        out[:, :, t] = x
    return out


def _build_program():
    import concourse.bacc as bacc
    import concourse.mybir as mybir
    from concourse.tile import TileContext
    from concourse.ap import AP

    f32 = mybir.dt.float32
    f8 = mybir.dt.float8e4
    ident = mybir.ActivationFunctionType.Identity
    DR = mybir.MatmulPerfMode.DoubleRow
    SC = _S_OUT / _S_H
    HALF = _SEQ // 2                 # 2048
    SLOT = _SEQ                      # ring slot stride (elements)

    nc = bacc.Bacc()
    f_d = nc.declare_dram_parameter("f", [_T, _SEQ], f8, isOutput=False)
    # [H0T|H1T] and [H1T|H0T] stacked along free dim (DoubleRow k-tiles)
    w01_d = nc.declare_dram_parameter("w01", [_P, 2 * _P], f8, isOutput=False)
    w10_d = nc.declare_dram_parameter("w10", [_P, 2 * _P], f8, isOutput=False)
    out_d = nc.declare_dram_parameter("out", [_T, _SEQ], f8, isOutput=True)

    with TileContext(nc) as tc:
        with tc.tile_pool(name="const", bufs=1) as cpool, \
             tc.tile_pool(name="oout", bufs=4) as opool, \
             tc.tile_pool(name="ps", bufs=4, space="PSUM") as pspool:
            w01_t = cpool.tile([_P, 2 * _P], f8, tag="w01")
            nc.sync.dma_start(out=w01_t[:], in_=w01_d[:, :])
            w10_t = cpool.tile([_P, 2 * _P], f8, tag="w10")
            nc.sync.dma_start(out=w10_t[:], in_=w10_d[:, :])
            w01 = w01_t[:, :].rearrange("p (two m) -> p two m", two=2)
            w10 = w10_t[:, :].rearrange("p (two m) -> p two m", two=2)

            # 4-slot input ring: (cur, prev) blocks at fixed free-dim offsets
            NRING = 4
            fring = cpool.tile([_P, NRING * SLOT], f8, tag="fring")

            QW = _SEQ // 4           # 1024-wide psum quarter tiles
            for k in range(_NBLK):
                rows = slice(k * _P, (k + 1) * _P)
                s_cur = (k % NRING) * SLOT
                s_prev = ((k - 1) % NRING) * SLOT
                # input loads on SWDGE (GpSimd) - leaves ACT free for evicts
                nc.gpsimd.dma_start(out=fring[:, s_cur:s_cur + SLOT],
                                    in_=f_d[rows, :])
                # ascending (base, base+stride) pair selects which weight
                # order pairs with (cur, prev) in the DoubleRow contraction
                if s_cur < s_prev:
                    base, stride, w3d = s_cur, s_prev - s_cur, w01
                else:
                    base, stride, w3d = s_prev, s_cur - s_prev, w10
                proto = fring[:, 0:1]
                o_t = opool.tile([_P, _SEQ], f8, tag="o")
                for q in range(4):
                    qb = q * QW
                    ps = pspool.tile([_P, QW], f32, tag="ps")
                    for c in range(2):
                        sl = slice(qb + c * 512, qb + (c + 1) * 512)
                        if k == 0:
                            # no previous block: plain matmul on H0T only
                            nc.tensor.matmul(ps[:, c * 512:(c + 1) * 512],
                                             w01_t[:, 0:_P],
                                             fring[:, sl],
                                             start=True, stop=True)
                        else:
                            off = base + qb + (c * 512)
                            rhs = AP(proto.tensor, proto.offset + off,
                                     [list(proto.ap[0]), [stride, 2],
                                      [1, 512]])
                            nc.tensor.matmul(ps[:, c * 512:(c + 1) * 512],
                                             w3d, rhs,
                                             start=True, stop=True,
                                             perf_mode=DR)
                    # evict PSUM->SBUF with the fp8 rescale; one op per
                    # quarter, alternating the otherwise-idle ACT / DVE
                    if q % 2 == 0:
                        nc.scalar.activation(o_t[:, qb:qb + QW],
                                             ps[:, :], ident,
                                             bias=0.0, scale=SC)
                    else:
                        nc.vector.tensor_scalar_mul(o_t[:, qb:qb + QW],
                                                    ps[:, :], SC)
                    if q % 2 == 1:
                        hb = qb - QW
                        nc.sync.dma_start(out=out_d[rows, hb:hb + HALF],
                                          in_=o_t[:, hb:hb + HALF])
    nc.compile()
    return nc


def kernel(force, goal, ax, bx):
    global LAST_RESULT
    import ml_dtypes

    force = np.asarray(force, dtype=np.float32)
    goal = np.asarray(goal, dtype=np.float32)
    assert force.shape == (_B, _N, _T), force.shape

    h = _impulse(float(ax), float(bx), _T)
    # Fast path needs the taps beyond 128 to be negligible.
    hn = np.linalg.norm(h)
    if not np.isfinite(hn) or hn == 0.0 or \
            np.linalg.norm(h[_P:]) / hn > 1e-3:
        return _kernel_numpy(force, goal, ax, bx)

    f8 = ml_dtypes.float8_e4m3fn

    # Filter matrices (lhsT layout: [K=i(time-in), M=j(time-out)]).
    idx_j, idx_i = np.meshgrid(np.arange(_P), np.arange(_P), indexing="xy")
    # h0t[i, j] = h[j-i] for j>=i      (intra-block taps 0..127)
    lag0 = idx_j - idx_i
    h0t = np.where(lag0 >= 0, h[np.clip(lag0, 0, _P - 1)], 0.0) * _S_H
    # h1t[i, j] = h[j+128-i] for lag<128 (previous-block taps 1..127)
    lag1 = idx_j + _P - idx_i
    h1t = np.where(lag1 < _P, h[np.clip(lag1, 0, _P - 1)], 0.0) * _S_H
    h0t = np.ascontiguousarray(h0t, dtype=np.float32).astype(f8)
    h1t = np.ascontiguousarray(h1t, dtype=np.float32).astype(f8)
    w01 = np.ascontiguousarray(np.concatenate([h0t, h1t], axis=1))
    w10 = np.ascontiguousarray(np.concatenate([h1t, h0t], axis=1))

    nc = _build_program()

    # Shard: core c gets batches [256c, 256c+256) -> [T, SEQ] fp8, transposed
    fq = force.reshape(_NCORES, _SEQ, _T).astype(f8)
    in_maps = [
        {
            "f": np.ascontiguousarray(fq[c].T),
            "w01": w01,
            "w10": w10,
        }
        for c in range(_NCORES)
    ]

    from concourse.bass_utils import run_bass_kernel_spmd
    res = run_bass_kernel_spmd(
        nc, in_maps, list(range(_NCORES)),
        trace=bool(os.environ.get("KERNEL_TRACE")),
    )
    LAST_RESULT = res

    # Host reconstruction: force part (device) + rank-1 goal part (exact).
    g = (float(ax) * float(bx)) * np.cumsum(h)          # (T,) fp64
    out = np.empty((_B, _N, _T), dtype=np.float32)
    ov = out.reshape(_NCORES, _SEQ, _T)
    inv = np.float32(1.0 / _S_OUT)
    gp32 = g.astype(np.float32)
    goal_v = goal.reshape(_NCORES, _SEQ)
    for c in range(_NCORES):
        dev = res.results[c]["out"].astype(np.float32).T   # (SEQ, T)
        np.multiply(dev, inv, out=dev)
        dev += goal_v[c][:, None] * gp32[None, :]
        ov[c] = dev
    return out


# revision 11
# speedup vs baseline: 1.1838x; 1.1001x over previous
"""Trainium2 kernel for the damped-spring (DMP-style) batched scan.

Reference semantics (per batch b, dof n, x0=dx0=0):
    ddx_t = ax*(bx*(goal - x_t) - dx_t) + f_t
    dx += ddx_t*DT;  x += dx*DT;  traj[..., t] = x

Linear time-invariant 2nd-order recurrence in s=(x,dx):
    s_{t+1} = A s_t + v*u_t,   u_t = f_t + ax*bx*goal,  v = (DT^2, DT)
so traj splits into two independent parts:
    traj[b,n,t] = conv(h, force[b,n,:])[t]  +  goal[b,n] * g(t)
with h(k) = [A^k v]_0 the x-impulse response and g(t) = ax*bx*cumsum(h).
For ax=25, bx=6.25 the poles are 0.912/0.822, so h decays below 1e-5 of
its peak within 128 taps: the sequential scan becomes a SHORT CAUSAL
CONVOLUTION (128 taps), i.e. pure tensor-engine matmuls with no serial
dependency at all.

Device computes only the force part (99.9% of the FLOPs, ~0.1% of the
output norm), tiled per 128-time-block as
    psum[j,f] = sum_i H0[j,i] f_blk[i,f] + sum_i H1[j,i] f_prev[i,f]
with f pre-transposed to [T, F] on the host so time is the contraction
(partition) axis.  Both taps matrices are fused into ONE fp8 DoubleRow
matmul per 512-seq chunk (K=256 effective contraction, 2 fp8 weights per
PE cell): f blocks live in a 3-slot SBUF ring so (cur, prev) sit at a
fixed positive stride, with [H1|H0]-swapped weights covering the ring
wrap.  The exactly-known rank-1 goal part goal*g(t) is added back on the
host in fp64.  fp8 e4m3 in / fp8 out keeps DMA at 32 MB/core (vs 128 MB
for the fp32 scan baseline); fp32 PSUM accumulation keeps the end-to-end
relative L2 error ~6e-5 (tolerance 2e-2).  All values are kept inside
+-240 (TRN fp8e4 max normal).

Sharding: data-parallel over batch across 8 cores; core c takes batches
[256c, 256c+256) = 4096 sequences, each core fully independent.
"""

import os
import numpy as np

_B, _N, _T = 2048, 16, 4096
_NCORES = 8
_P = 128
_SEQ = (_B // _NCORES) * _N          # 4096 sequences per core
_NBLK = _T // _P                     # 32 time blocks
_DT = float(np.float32(0.01))
_S_H = float(2.0 ** 18)              # fp8 scale on the filter taps
_S_OUT = float(2.0 ** 8)             # fp8 scale on the output (force part)

LAST_RESULT = None                   # BassKernelResults stash for harnesses


def _impulse(ax: float, bx: float, n: int):
    """fp64 impulse response h[k] = [A^k v]_0 of the discrete recurrence."""
    a, b, dt = float(ax), float(bx), _DT
    A = np.array(
        [[1.0 - a * b * dt * dt, dt * (1.0 - a * dt)],
         [-a * b * dt, 1.0 - a * dt]], dtype=np.float64)
    v = np.array([dt * dt, dt], dtype=np.float64)
    h = np.empty(n, dtype=np.float64)
    w = v.copy()
    for k in range(n):
        h[k] = w[0]
        w = A @ w
    return h


def _kernel_numpy(force, goal, ax, bx):
    """Exact fallback (slow): used only if the taps don't decay fast."""
    B, N, T = force.shape
    dt = np.float32(_DT)
    x = np.zeros((B, N), np.float32)
    dx = np.zeros((B, N), np.float32)
    out = np.empty((B, N, T), np.float32)
    axf, bxf = np.float32(ax), np.float32(bx)
    for t in range(T):
        ddx = axf * (bxf * (goal - x) - dx) + force[:, :, t]
        dx = dx + ddx * dt
        x = x + dx * dt
        out[:, :, t] = x
    return out


def _build_program():
    import concourse.bacc as bacc
    import concourse.mybir as mybir
    from concourse.tile import TileContext
    from concourse.ap import AP

    f32 = mybir.dt.float32
    f8 = mybir.dt.float8e4
    ident = mybir.ActivationFunctionType.Identity
    DR = mybir.MatmulPerfMode.DoubleRow
    SC = _S_OUT / _S_H
    HALF = _SEQ // 2                 # 2048
    SLOT = _SEQ                      # ring slot stride (elements)

    nc = bacc.Bacc()
    f_d = nc.declare_dram_parameter("f", [_T, _SEQ], f8, isOutput=False)
    # [H0T|H1T] and [H1T|H0T] stacked along free dim (DoubleRow k-tiles)
    w01_d = nc.declare_dram_parameter("w01", [_P, 2 * _P], f8, isOutput=False)
    w10_d = nc.declare_dram_parameter("w10", [_P, 2 * _P], f8, isOutput=False)
    out_d = nc.declare_dram_parameter("out", [_T, _SEQ], f8, isOutput=True)

    with TileContext(nc) as tc:
        with tc.tile_pool(name="const", bufs=1) as cpool, \
             tc.tile_pool(name="oout", bufs=6) as opool, \
             tc.tile_pool(name="ps", bufs=4, space="PSUM") as pspool:
            w01_t = cpool.tile([_P, 2 * _P], f8, tag="w01")
            nc.sync.dma_start(out=w01_t[:], in_=w01_d[:, :])
            w10_t = cpool.tile([_P, 2 * _P], f8, tag="w10")
            nc.sync.dma_start(out=w10_t[:], in_=w10_d[:, :])
            w01 = w01_t[:, :].rearrange("p (two m) -> p two m", two=2)
            w10 = w10_t[:, :].rearrange("p (two m) -> p two m", two=2)

            # 4-slot input ring: (cur, prev) blocks at fixed free-dim offsets
            NRING = 6
            fring = cpool.tile([_P, NRING * SLOT], f8, tag="fring")

            QW = _SEQ // 4           # 1024-wide psum quarter tiles
            for k in range(_NBLK):
                rows = slice(k * _P, (k + 1) * _P)
                s_cur = (k % NRING) * SLOT
                s_prev = ((k - 1) % NRING) * SLOT
                # input loads on SWDGE (GpSimd) - leaves ACT free for evicts
                nc.gpsimd.dma_start(out=fring[:, s_cur:s_cur + SLOT],
                                    in_=f_d[rows, :])
                # ascending (base, base+stride) pair selects which weight
                # order pairs with (cur, prev) in the DoubleRow contraction
                if s_cur < s_prev:
                    base, stride, w3d = s_cur, s_prev - s_cur, w01
                else:
                    base, stride, w3d = s_prev, s_cur - s_prev, w10
                proto = fring[:, 0:1]
                o_t = opool.tile([_P, _SEQ], f8, tag="o")
                for q in range(4):
                    qb = q * QW
                    ps = pspool.tile([_P, QW], f32, tag="ps")
                    for c in range(2):
                        sl = slice(qb + c * 512, qb + (c + 1) * 512)
                        if k == 0:
                            # no previous block: plain matmul on H0T only
                            nc.tensor.matmul(ps[:, c * 512:(c + 1) * 512],
                                             w01_t[:, 0:_P],
                                             fring[:, sl],
                                             start=True, stop=True)
                        else:
                            off = base + qb + (c * 512)
                            rhs = AP(proto.tensor, proto.offset + off,
                                     [list(proto.ap[0]), [stride, 2],
                                      [1, 512]])
                            nc.tensor.matmul(ps[:, c * 512:(c + 1) * 512],
                                             w3d, rhs,
                                             start=True, stop=True,
                                             perf_mode=DR)
                    # evict PSUM->SBUF with the fp8 rescale; one op per
                    # quarter, alternating the otherwise-idle ACT / DVE
                    if q % 2 == 0:
                        nc.scalar.activation(o_t[:, qb:qb + QW],
                                             ps[:, :], ident,
                                             bias=0.0, scale=SC)
                    else:
                        nc.vector.tensor_scalar_mul(o_t[:, qb:qb + QW],
                                                    ps[:, :], SC)
                    if q % 2 == 1:
                        hb = qb - QW
                        nc.sync.dma_start(out=out_d[rows, hb:hb + HALF],
                                          in_=o_t[:, hb:hb + HALF])
    nc.compile()
    return nc


def kernel(force, goal, ax, bx):
    global LAST_RESULT
    import ml_dtypes

    force = np.asarray(force, dtype=np.float32)
    goal = np.asarray(goal, dtype=np.float32)
    assert force.shape == (_B, _N, _T), force.shape

    h = _impulse(float(ax), float(bx), _T)
    # Fast path needs the taps beyond 128 to be negligible.
    hn = np.linalg.norm(h)
    if not np.isfinite(hn) or hn == 0.0 or \
            np.linalg.norm(h[_P:]) / hn > 1e-3:
        return _kernel_numpy(force, goal, ax, bx)

    f8 = ml_dtypes.float8_e4m3fn

    # Filter matrices (lhsT layout: [K=i(time-in), M=j(time-out)]).
    idx_j, idx_i = np.meshgrid(np.arange(_P), np.arange(_P), indexing="xy")
    # h0t[i, j] = h[j-i] for j>=i      (intra-block taps 0..127)
    lag0 = idx_j - idx_i
    h0t = np.where(lag0 >= 0, h[np.clip(lag0, 0, _P - 1)], 0.0) * _S_H
    # h1t[i, j] = h[j+128-i] for lag<128 (previous-block taps 1..127)
    lag1 = idx_j + _P - idx_i
    h1t = np.where(lag1 < _P, h[np.clip(lag1, 0, _P - 1)], 0.0) * _S_H
    h0t = np.ascontiguousarray(h0t, dtype=np.float32).astype(f8)
    h1t = np.ascontiguousarray(h1t, dtype=np.float32).astype(f8)
    w01 = np.ascontiguousarray(np.concatenate([h0t, h1t], axis=1))
    w10 = np.ascontiguousarray(np.concatenate([h1t, h0t], axis=1))

    nc = _build_program()

    # Shard: core c gets batches [256c, 256c+256) -> [T, SEQ] fp8, transposed
    fq = force.reshape(_NCORES, _SEQ, _T).astype(f8)
    in_maps = [
        {
            "f": np.ascontiguousarray(fq[c].T),
            "w01": w01,
            "w10": w10,
        }
        for c in range(_NCORES)
    ]

    from concourse.bass_utils import run_bass_kernel_spmd
    res = run_bass_kernel_spmd(
        nc, in_maps, list(range(_NCORES)),
        trace=bool(os.environ.get("KERNEL_TRACE")),
    )
    LAST_RESULT = res

    # Host reconstruction: force part (device) + rank-1 goal part (exact).
    g = (float(ax) * float(bx)) * np.cumsum(h)          # (T,) fp64
    out = np.empty((_B, _N, _T), dtype=np.float32)
    ov = out.reshape(_NCORES, _SEQ, _T)
    inv = np.float32(1.0 / _S_OUT)
    gp32 = g.astype(np.float32)
    goal_v = goal.reshape(_NCORES, _SEQ)
    for c in range(_NCORES):
        dev = res.results[c]["out"].astype(np.float32).T   # (SEQ, T)
        np.multiply(dev, inv, out=dev)
        dev += goal_v[c][:, None] * gp32[None, :]
        ov[c] = dev
    return out


# revision 12
# speedup vs baseline: 1.2098x; 1.0220x over previous
"""Trainium2 kernel for the damped-spring (DMP-style) batched scan.

Reference semantics (per batch b, dof n, x0=dx0=0):
    ddx_t = ax*(bx*(goal - x_t) - dx_t) + f_t
    dx += ddx_t*DT;  x += dx*DT;  traj[..., t] = x

Linear time-invariant 2nd-order recurrence in s=(x,dx):
    s_{t+1} = A s_t + v*u_t,   u_t = f_t + ax*bx*goal,  v = (DT^2, DT)
so traj splits into two independent parts:
    traj[b,n,t] = conv(h, force[b,n,:])[t]  +  goal[b,n] * g(t)
with h(k) = [A^k v]_0 the x-impulse response and g(t) = ax*bx*cumsum(h).
For ax=25, bx=6.25 the poles are 0.912/0.822, so h decays below 1e-5 of
its peak within 128 taps: the sequential scan becomes a SHORT CAUSAL
CONVOLUTION (128 taps), i.e. pure tensor-engine matmuls with no serial
dependency at all.

Device computes only the force part (99.9% of the FLOPs, ~0.1% of the
output norm), tiled per 128-time-block as
    psum[j,f] = sum_i H0[j,i] f_blk[i,f] + sum_i H1[j,i] f_prev[i,f]
with f pre-transposed to [T, F] on the host so time is the contraction
(partition) axis.  Both taps matrices are fused into ONE fp8 DoubleRow
matmul per 512-seq chunk (K=256 effective contraction, 2 fp8 weights per
PE cell): f blocks live in a 3-slot SBUF ring so (cur, prev) sit at a
fixed positive stride, with [H1|H0]-swapped weights covering the ring
wrap.  The exactly-known rank-1 goal part goal*g(t) is added back on the
host in fp64.  fp8 e4m3 in / fp8 out keeps DMA at 32 MB/core (vs 128 MB
for the fp32 scan baseline); fp32 PSUM accumulation keeps the end-to-end
relative L2 error ~6e-5 (tolerance 2e-2).  All values are kept inside
+-240 (TRN fp8e4 max normal).

Sharding: data-parallel over batch across 8 cores; core c takes batches
[256c, 256c+256) = 4096 sequences, each core fully independent.
"""

import os
import numpy as np

_B, _N, _T = 2048, 16, 4096
_NCORES = 8
_P = 128
_SEQ = (_B // _NCORES) * _N          # 4096 sequences per core
_NBLK = _T // _P                     # 32 time blocks
_DT = float(np.float32(0.01))
_S_H = float(2.0 ** 18)              # fp8 scale on the filter taps
_S_OUT = float(2.0 ** 8)             # fp8 scale on the output (force part)

LAST_RESULT = None                   # BassKernelResults stash for harnesses


def _impulse(ax: float, bx: float, n: int):
    """fp64 impulse response h[k] = [A^k v]_0 of the discrete recurrence."""
    a, b, dt = float(ax), float(bx), _DT
    A = np.array(
        [[1.0 - a * b * dt * dt, dt * (1.0 - a * dt)],
         [-a * b * dt, 1.0 - a * dt]], dtype=np.float64)
    v = np.array([dt * dt, dt], dtype=np.float64)
    h = np.empty(n, dtype=np.float64)
    w = v.copy()
    for k in range(n):
        h[k] = w[0]
        w = A @ w
    return h


def _kernel_numpy(force, goal, ax, bx):
    """Exact fallback (slow): used only if the taps don't decay fast."""
    B, N, T = force.shape
    dt = np.float32(_DT)
    x = np.zeros((B, N), np.float32)
    dx = np.zeros((B, N), np.float32)
    out = np.empty((B, N, T), np.float32)
    axf, bxf = np.float32(ax), np.float32(bx)
    for t in range(T):
        ddx = axf * (bxf * (goal - x) - dx) + force[:, :, t]
        dx = dx + ddx * dt
        x = x + dx * dt
        out[:, :, t] = x
    return out


def _build_program():
    import concourse.bacc as bacc
    import concourse.mybir as mybir
    from concourse.tile import TileContext
    from concourse.ap import AP

    f32 = mybir.dt.float32
    f8 = mybir.dt.float8e4
    ident = mybir.ActivationFunctionType.Identity
    DR = mybir.MatmulPerfMode.DoubleRow
    SC = _S_OUT / _S_H
    HALF = _SEQ // 2                 # 2048
    SLOT = _SEQ                      # ring slot stride (elements)

    nc = bacc.Bacc()
    f_d = nc.declare_dram_parameter("f", [_T, _SEQ], f8, isOutput=False)
    # [H0T|H1T] and [H1T|H0T] stacked along free dim (DoubleRow k-tiles)
    w01_d = nc.declare_dram_parameter("w01", [_P, 2 * _P], f8, isOutput=False)
    w10_d = nc.declare_dram_parameter("w10", [_P, 2 * _P], f8, isOutput=False)
    out_d = nc.declare_dram_parameter("out", [_T, _SEQ], f8, isOutput=True)

    with TileContext(nc) as tc:
        with tc.tile_pool(name="const", bufs=1) as cpool, \
             tc.tile_pool(name="oout", bufs=6) as opool, \
             tc.tile_pool(name="ps", bufs=4, space="PSUM") as pspool:
            w01_t = cpool.tile([_P, 2 * _P], f8, tag="w01")
            nc.sync.dma_start(out=w01_t[:], in_=w01_d[:, :])
            w10_t = cpool.tile([_P, 2 * _P], f8, tag="w10")
            nc.sync.dma_start(out=w10_t[:], in_=w10_d[:, :])
            w01 = w01_t[:, :].rearrange("p (two m) -> p two m", two=2)
            w10 = w10_t[:, :].rearrange("p (two m) -> p two m", two=2)

            # SBUF holds the ENTIRE per-core input (128 KiB/partition):
            # no slot reuse, so every input DMA is issued upfront and the
            # input stream runs at full HBM rate, never compute-blocked.
            # prev block is always at cur-SLOT, so weights are uniformly
            # [H1|H0] with an ascending (prev, cur) stride of +SLOT.
            fring = cpool.tile([_P, _NBLK * SLOT], f8, tag="fring")
            proto = fring[:, 0:1]
            f_proto = f_d[0:_P, :]

            # fast ramp-in: row 0 in quarters, rows 1-3 single, then 2 MiB
            # 4-row blocks - all on SWDGE (GpSimd), leaving ACT free
            for c in range(4):
                nc.gpsimd.dma_start(
                    out=fring[:, c * 1024:(c + 1) * 1024],
                    in_=f_d[0:_P, c * 1024:(c + 1) * 1024])
            for k in range(1, 4):
                nc.gpsimd.dma_start(
                    out=fring[:, k * SLOT:(k + 1) * SLOT],
                    in_=f_d[k * _P:(k + 1) * _P, :])
            for k4 in range(1, _NBLK // 4):
                src = AP(f_proto.tensor, f_proto.offset + k4 * 4 * _P * _SEQ,
                         [list(f_proto.ap[0]), [_P * _SEQ, 4], [1, _SEQ]])
                dst = AP(proto.tensor, proto.offset + k4 * 4 * SLOT,
                         [list(proto.ap[0]), [SLOT, 4], [1, _SEQ]])
                nc.gpsimd.dma_start(out=dst, in_=src)

            QW = _SEQ // 4           # 1024-wide psum quarter tiles
            for k in range(_NBLK):
                rows = slice(k * _P, (k + 1) * _P)
                base = (k - 1) * SLOT
                o_t = opool.tile([_P, _SEQ], f8, tag="o")
                for q in range(4):
                    qb = q * QW
                    ps = pspool.tile([_P, QW], f32, tag="ps")
                    for c in range(2):
                        sl = slice(qb + c * 512, qb + (c + 1) * 512)
                        if k == 0:
                            # no previous block: plain matmul on H0T only
                            nc.tensor.matmul(ps[:, c * 512:(c + 1) * 512],
                                             w01_t[:, 0:_P],
                                             fring[:, sl],
                                             start=True, stop=True)
                        else:
                            off = base + qb + (c * 512)
                            rhs = AP(proto.tensor, proto.offset + off,
                                     [list(proto.ap[0]), [SLOT, 2],
                                      [1, 512]])
                            nc.tensor.matmul(ps[:, c * 512:(c + 1) * 512],
                                             w10, rhs,
                                             start=True, stop=True,
                                             perf_mode=DR)
                    # evict PSUM->SBUF with the fp8 rescale; one op per
                    # quarter, alternating the otherwise-idle ACT / DVE
                    if q % 2 == 0:
                        nc.scalar.activation(o_t[:, qb:qb + QW],
                                             ps[:, :], ident,
                                             bias=0.0, scale=SC)
                    else:
                        nc.vector.tensor_scalar_mul(o_t[:, qb:qb + QW],
                                                    ps[:, :], SC)
                    if q % 2 == 1:
                        hb = qb - QW
                        nc.sync.dma_start(out=out_d[rows, hb:hb + HALF],
                                          in_=o_t[:, hb:hb + HALF])
    nc.compile()
    return nc


def kernel(force, goal, ax, bx):
    global LAST_RESULT
    import ml_dtypes

    force = np.asarray(force, dtype=np.float32)
    goal = np.asarray(goal, dtype=np.float32)
    assert force.shape == (_B, _N, _T), force.shape

    h = _impulse(float(ax), float(bx), _T)
    # Fast path needs the taps beyond 128 to be negligible.
    hn = np.linalg.norm(h)
    if not np.isfinite(hn) or hn == 0.0 or \
            np.linalg.norm(h[_P:]) / hn > 1e-3:
        return _kernel_numpy(force, goal, ax, bx)

    f8 = ml_dtypes.float8_e4m3fn

    # Filter matrices (lhsT layout: [K=i(time-in), M=j(time-out)]).
    idx_j, idx_i = np.meshgrid(np.arange(_P), np.arange(_P), indexing="xy")
    # h0t[i, j] = h[j-i] for j>=i      (intra-block taps 0..127)
    lag0 = idx_j - idx_i
    h0t = np.where(lag0 >= 0, h[np.clip(lag0, 0, _P - 1)], 0.0) * _S_H
    # h1t[i, j] = h[j+128-i] for lag<128 (previous-block taps 1..127)
    lag1 = idx_j + _P - idx_i
    h1t = np.where(lag1 < _P, h[np.clip(lag1, 0, _P - 1)], 0.0) * _S_H
    h0t = np.ascontiguousarray(h0t, dtype=np.float32).astype(f8)
    h1t = np.ascontiguousarray(h1t, dtype=np.float32).astype(f8)
    w01 = np.ascontiguousarray(np.concatenate([h0t, h1t], axis=1))
    w10 = np.ascontiguousarray(np.concatenate([h1t, h0t], axis=1))

    nc = _build_program()

    # Shard: core c gets batches [256c, 256c+256) -> [T, SEQ] fp8, transposed
    fq = force.reshape(_NCORES, _SEQ, _T).astype(f8)
    in_maps = [
        {
            "f": np.ascontiguousarray(fq[c].T),
            "w01": w01,
            "w10": w10,
        }
        for c in range(_NCORES)
    ]

    from concourse.bass_utils import run_bass_kernel_spmd
    res = run_bass_kernel_spmd(
        nc, in_maps, list(range(_NCORES)),
        trace=bool(os.environ.get("KERNEL_TRACE")),
    )
    LAST_RESULT = res

    # Host reconstruction: force part (device) + rank-1 goal part (exact).
    g = (float(ax) * float(bx)) * np.cumsum(h)          # (T,) fp64
    out = np.empty((_B, _N, _T), dtype=np.float32)
    ov = out.reshape(_NCORES, _SEQ, _T)
    inv = np.float32(1.0 / _S_OUT)
    gp32 = g.astype(np.float32)
    goal_v = goal.reshape(_NCORES, _SEQ)
    for c in range(_NCORES):
        dev = res.results[c]["out"].astype(np.float32).T   # (SEQ, T)
        np.multiply(dev, inv, out=dev)
        dev += goal_v[c][:, None] * gp32[None, :]
        ov[c] = dev
    return out
